# revision 1
# baseline (speedup 1.0000x reference)
"""DenseNGCN layer on 8 trn2 NeuronCores.

  x = features @ weight                    [50000, 512] @ [512, 64]
  x = A @ x   (twice, A sparse COO E=800k: segment_sum(val * x[col], row))
  out = x + bias

Strategy (dst-node sharding, per the sharding hint):
  - Nodes sharded across 8 cores (6250 rows each, padded to 6272 = 49 blocks
    of 128). Each core owns the projection + SpMM rows of its shard.
  - Edges partitioned by destination row. Per 128-row dst block, edges are
    padded into chunks of 128; per chunk the kernel
      * dma_gathers the 128 source rows x[col] (one 256 B descriptor per
        edge) from the core-local replica of x,
      * loads a host-precomputed one-hot scatter matrix
        S[p, j] = val_p * (rowlocal_p == j) (bf16, dense from HBM),
      * accumulates psum += S.T @ G on the tensor engine,
    which turns the segment-sum into dense matmuls.
  - x is stored as bf16 [N, 64]; rows are gathered in PAIRS (elem = 256 B =
    2 rows) with edges grouped by source parity, so the pair index fits the
    gather's int16 index type and each chunk reads a compile-time half of
    the gathered pair.
  - Full x is re-replicated to every core's HBM between SpMM iterations via
    an AllGather collective (bf16: 6.4 MB).

All edge metadata (sorted/padded pair indices and the dense S tensor) is
precomputed host-side into per-core tensors; the device program is identical
across cores (SPMD).
"""

import numpy as np
import ml_dtypes

N = 50000
E = 800000
IN_CH = 512
OUT_CH = 64
C = 8
P = 128
NSHARD = N // C                # 6250
BLKS = (NSHARD + P - 1) // P   # 49
NPAD = BLKS * P                # 6272
NTOT = NPAD * C                # 50176

_CACHE = {}


def _prep(adj_indices, adj_values):
    """Sort/pad edges into per-(core, dst-block) chunk grids; build gather
    pair-indices and the dense bf16 one-hot S tensor."""
    row = adj_indices[0].astype(np.int64)
    col = adj_indices[1].astype(np.int64)
    val = adj_values.astype(np.float32)

    core = row // NSHARD
    loc = row % NSHARD
    blk = loc // P
    rl = loc % P
    pcol = (col // NSHARD) * NPAD + (col % NSHARD)
    parity = pcol & 1

    key = (core * BLKS + blk) * 2 + parity
    order = np.argsort(key, kind="stable")
    counts = np.bincount(key, minlength=C * BLKS * 2)

    CE = int(-(-counts[0::2].max() // P))
    CO = int(-(-counts[1::2].max() // P))
    KCH = CE + CO
    SLOTS = KCH * P

    start = np.zeros_like(counts)
    start[1:] = np.cumsum(counts)[:-1]
    rank = np.arange(E) - start[key[order]]
    par_s = parity[order]
    slot = rank + par_s * (CE * P)          # slot within the block grid
    c_s = core[order]
    b_s = blk[order]
    ch_s = slot // P                        # chunk within block
    pe_s = slot % P                         # edge partition within chunk

    # gather pair-index grid [C, BLKS, SLOTS]
    g_idx = np.zeros((C, BLKS, SLOTS), np.int64)
    g_idx[c_s, b_s, slot] = pcol[order] >> 1

    # wrap: logical i -> [i%16, i//16], one gather call per block
    gi = g_idx.reshape(C, BLKS, KCH * 8, 16)
    gi = gi.transpose(0, 3, 1, 2).reshape(C, 16, BLKS * KCH * 8)
    gall = np.tile(gi, (1, 8, 1)).astype(np.int16)

    # dense one-hot S, partition-major: s[c][pe, (b*KCH+ch)*128 + rl] = val
    s_host = np.zeros((C, P, BLKS * KCH * P), ml_dtypes.bfloat16)
    flat_col = (b_s * KCH + ch_s) * P + rl[order]
    s_host[c_s, pe_s, flat_col] = val[order].astype(ml_dtypes.bfloat16)

    return CE, CO, gall, s_host


def _build(CE, CO):
    import concourse.bacc as bacc
    import concourse.mybir as mybir
    from concourse import tile

    f32 = mybir.dt.float32
    bf16 = mybir.dt.bfloat16
    i16 = mybir.dt.int16
    KCH = CE + CO

    nc = bacc.Bacc(
        None,
        target_bir_lowering=False,
        num_devices=C,
        dynamic_dma_scratch_size=1 << 16,
    )

    featT_d = nc.dram_tensor("featT", [IN_CH, NPAD], bf16, kind="ExternalInput")
    w_d = nc.dram_tensor("w", [IN_CH, OUT_CH], bf16, kind="ExternalInput")
    bias_d = nc.dram_tensor("bias", [P, OUT_CH], f32, kind="ExternalInput")
    gi_d = nc.dram_tensor("gi", [P, BLKS * KCH * 8], i16, kind="ExternalInput")
    s_d = nc.dram_tensor("s", [P, BLKS * KCH * P], bf16, kind="ExternalInput")
    out_d = nc.dram_tensor("out", [NPAD, OUT_CH], f32, kind="ExternalOutput")

    xsh_d = nc.dram_tensor("x_shard", [NPAD, OUT_CH], bf16)
    xA_d = nc.dram_tensor("xA", [NTOT, OUT_CH], bf16)
    xB_d = nc.dram_tensor("xB", [NTOT, OUT_CH], bf16)

    with tile.TileContext(nc) as tc:
        with (
            tc.tile_pool(name="const", bufs=1) as cpool,
            tc.tile_pool(name="g", bufs=3) as gpool,
            tc.tile_pool(name="s", bufs=3) as spool,
            tc.tile_pool(name="o", bufs=3) as opool,
            tc.tile_pool(name="psum", bufs=4, space="PSUM") as pp,
        ):
            w_sb = cpool.tile([P, IN_CH // P, OUT_CH], bf16)
            bias_sb = cpool.tile([P, OUT_CH], f32)
            gi_sb = cpool.tile([P, BLKS * KCH * 8], i16)
            x_sb = cpool.tile([P, BLKS, OUT_CH], bf16)

            nc.sync.dma_start(w_sb[:], w_d[:].rearrange("(k p) c -> p k c", p=P))
            nc.sync.dma_start(bias_sb[:], bias_d[:])
            nc.sync.dma_start(gi_sb[:], gi_d[:])

            # --- projection: x0 = features @ W for this core's rows ---
            GRP = 7  # blocks per feature-tile group (49 = 7*7)
            with tc.tile_pool(name="feat", bufs=2) as fpool:
                for g in range(BLKS // GRP):
                    feat_sb = fpool.tile([P, IN_CH // P, GRP * P], bf16, tag="f")
                    nc.sync.dma_start(
                        feat_sb[:],
                        featT_d[:, g * GRP * P : (g + 1) * GRP * P].rearrange(
                            "(k p) n -> p k n", p=P
                        ),
                    )
                    for bb in range(GRP):
                        b = g * GRP + bb
                        ps = pp.tile([P, OUT_CH], f32, tag="ps")
                        for k in range(IN_CH // P):
                            nc.tensor.matmul(
                                ps[:],
                                feat_sb[:, k, bb * P : (bb + 1) * P],
                                w_sb[:, k, :],
                                start=(k == 0),
                                stop=(k == IN_CH // P - 1),
                            )
                        nc.vector.tensor_copy(x_sb[:, b, :], ps[:])
                nc.sync.dma_start(
                    xsh_d[:].rearrange("(b p) c -> p b c", p=P), x_sb[:]
                )

            def allgather(dst):
                nc.gpsimd.collective_compute(
                    "AllGather",
                    mybir.AluOpType.bypass,
                    replica_groups=[list(range(C))],
                    ins=[xsh_d[:]],
                    outs=[dst[:]],
                )

            def spmm(src, last):
                # pair view: row i = x[2i] ++ x[2i+1], 256 B
                src_pairs = src[:].rearrange("(a b) c -> a (b c)", b=2)
                for b in range(BLKS):
                    G = gpool.tile([P, KCH, 2 * OUT_CH], bf16, tag="G")
                    nc.gpsimd.dma_gather(
                        G[:],
                        src_pairs,
                        gi_sb[:, b * KCH * 8 : (b + 1) * KCH * 8],
                        KCH * P,
                        KCH * P,
                        2 * OUT_CH,
                        single_packet=False,
                    )
                    S = spool.tile([P, KCH * P], bf16, tag="S")
                    nc.sync.dma_start(
                        S[:], s_d[:, b * KCH * P : (b + 1) * KCH * P]
                    )
                    ps = pp.tile([P, OUT_CH], f32, tag="ps")
                    for c in range(KCH):
                        off = 0 if c < CE else OUT_CH
                        nc.tensor.matmul(
                            ps[:],
                            S[:, c * P : (c + 1) * P],
                            G[:, c, off : off + OUT_CH],
                            start=(c == 0),
                            stop=(c == KCH - 1),
                        )
                    if last:
                        o = opool.tile([P, OUT_CH], f32, tag="o")
                        nc.vector.tensor_tensor(
                            o[:], ps[:], bias_sb[:], mybir.AluOpType.add
                        )
                        nc.sync.dma_start(out_d[b * P : (b + 1) * P, :], o[:])
                    else:
                        nc.vector.tensor_copy(x_sb[:, b, :], ps[:])
                if not last:
                    nc.sync.dma_start(
                        xsh_d[:].rearrange("(b p) c -> p b c", p=P), x_sb[:]
                    )

            allgather(xA_d)
            spmm(xA_d, last=False)
            allgather(xB_d)
            spmm(xB_d, last=True)

    nc.compile()
    return nc


LAST_RESULT = None


def kernel(adj_indices, adj_values, features, weight, bias):
    global LAST_RESULT
    from concourse.bass_utils import run_bass_kernel_spmd

    CE, CO, gall, s_host = _prep(
        np.asarray(adj_indices), np.asarray(adj_values)
    )

    key = (CE, CO)
    if key not in _CACHE:
        _CACHE[key] = _build(CE, CO)
    nc = _CACHE[key]

    features = np.asarray(features, np.float32)
    weight = np.ascontiguousarray(
        np.asarray(weight, np.float32).astype(ml_dtypes.bfloat16)
    )
    bias128 = np.tile(np.asarray(bias, np.float32).reshape(1, OUT_CH), (P, 1))

    in_maps = []
    for c in range(C):
        featT = np.zeros((IN_CH, NPAD), ml_dtypes.bfloat16)
        featT[:, :NSHARD] = (
            features[c * NSHARD : (c + 1) * NSHARD].T.astype(ml_dtypes.bfloat16)
        )
        in_maps.append(
            {
                "featT": featT,
                "w": weight,
                "bias": bias128,
                "gi": np.ascontiguousarray(gall[c]),
                "s": s_host[c],
            }
        )

    res = run_bass_kernel_spmd(nc, in_maps, core_ids=list(range(C)))
    LAST_RESULT = res

    out = np.concatenate(
        [res.results[c]["out"][:NSHARD] for c in range(C)], axis=0
    )
    return out



# revision 2
# speedup vs baseline: 1.0395x; 1.0395x over previous
"""DenseNGCN on 8 trn2 cores — mailbox (two-level one-hot matmul) design.

  x = features @ W; twice: x <- A @ x; out = x + bias.

Per pass, per core (dst-sharded, x replicated via AllGather in p-major
layout [128, 50, 64] per core):
  Stage A: packed mailbox of 160k slots (400 gsb-regions x 400: slot =
    gsb*400 + t*50 + bl, T=8): 1250 chunk matmuls (+~300 straddle splits)
    route source rows via fp8 0/1 Q one-hots streamed from HBM; psum
    chunks land contiguously in an HBM arena.
  Stage B: per dst block bl: strided DMA pulls [p = g0*8+t, G1] chunks
    (stride 50 rows); fp8 0/1 S one-hots streamed; vals applied to G via
    one batched broadcast multiply per 4 blocks; 25+2*OV matmuls per block
    accumulate in PSUM.
  Overflow edges beyond T=8 per (gsb, bl): one dma_gather of 256B pairs
    (b-adjacent rows in p-major layout) per pass feeds 2*OV extra matmuls
    per block (parity-split halves).
"""

import numpy as np
import ml_dtypes

N = 50000
E = 800000
IN_CH = 512
OUT_CH = 64
C = 8
P = 128
NSHARD = 6250
BLKS = 49            # real dst blocks per core
BLKP = 50            # padded p-major slab blocks (b=49 dummy)
NPAD = 6272
GSBP = C * BLKP      # 400 padded global src blocks
T = 8
RSZ = 512            # slots per gsb region (r = t*64 + bl)
NSLOT = GSBP * RSZ   # 160000
NCHA = NSLOT // P    # 1250 stage-A chunks
NCH_B = GSBP // 16   # 25
QB = 4               # stage-B blocks per load batch
NBP = (BLKS + QB - 1) // QB
AIT = 16             # stage-A chunks per iteration
QIT = 32             # chunks per Q load

_CACHE = {}


def _prep(adj_indices, adj_values):
    row = adj_indices[0].astype(np.int64)
    col = adj_indices[1].astype(np.int64)
    val = adj_values.astype(np.float32)

    cd = row // NSHARD
    bl = (row % NSHARD) // P
    dl = (row % NSHARD) % P
    cs = col // NSHARD
    lsrc = col % NSHARD
    bsrc = lsrc // P
    sl = lsrc % P
    gsb = cs * BLKP + bsrc

    percore = []
    max_ovf = 0
    for c in range(C):
        m = np.where(cd == c)[0]
        order = m[np.lexsort((m, bl[m], gsb[m]))]
        pg = gsb[order] * BLKS + bl[order]
        _, start_idx, cnt = np.unique(pg, return_index=True,
                                      return_counts=True)
        rank = np.arange(len(order)) - np.repeat(start_idx, cnt)
        main = rank < T
        ovf_bl = np.bincount(bl[order[~main]], minlength=BLKS)
        max_ovf = max(max_ovf, int(ovf_bl.max()))
        percore.append((order, rank, main))
    OV = max(1, (max_ovf + P - 1) // P)
    NST = NCH_B + 2 * OV

    f8 = ml_dtypes.float8_e4m3
    bf = ml_dtypes.bfloat16
    cores = []
    for c in range(C):
        order, rank, main = percore[c]
        e_main = order[main]
        t_main = rank[main]

        # packed Q: [128, NSLOT] fp8; slot = gsb*400 + t*50 + bl
        slot = gsb[e_main] * RSZ + t_main * 64 + bl[e_main]
        q = np.zeros((P, NSLOT), f8)
        q[sl[e_main], slot] = 1.0

        # S: [128, BLKS*NST*128] fp8 0/1; partition p = g0*8+t; col = dl
        s = np.zeros((P, BLKS * NST * P), f8)
        vgq = np.zeros((P, NBP * NCH_B * QB), bf)
        vo = np.zeros((P, BLKS * OV), bf)
        g0 = gsb[e_main] % 16
        G1 = gsb[e_main] // 16
        p_b = g0 * 8 + t_main
        s[p_b, (bl[e_main] * NST + G1) * P + dl[e_main]] = 1.0
        # vgq layout: (p, bp, G1, i)
        vgq[p_b, ((bl[e_main] // QB) * NCH_B + G1) * QB
            + bl[e_main] % QB] = val[e_main]

        e_ovf = order[~main]
        o_order = e_ovf[np.argsort(bl[e_ovf], kind="stable")]
        ob = bl[o_order]
        _, ostart, ocnt = np.unique(ob, return_index=True, return_counts=True)
        orank = np.zeros(len(o_order), np.int64)
        if len(o_order):
            orank = np.arange(len(o_order)) - np.repeat(ostart, ocnt)
        ch = ob * OV + orank // P
        pp_ = orank % P
        gi_ovf = np.zeros((BLKS * OV, P), np.int64)
        gi_ovf[ch, pp_] = ((cs[o_order] * P + sl[o_order]) * (BLKP // 2)
                           + bsrc[o_order] // 2)
        par = bsrc[o_order] % 2
        se = (ob * NST + NCH_B + 2 * (orank // P)) * P
        s[pp_[par == 0], se[par == 0] + dl[o_order[par == 0]]] = 1.0
        s[pp_[par == 1], se[par == 1] + P + dl[o_order[par == 1]]] = 1.0
        vo[pp_, ch] = val[o_order]

        nidx = BLKS * OV * P
        wrap = gi_ovf.reshape(-1).reshape(nidx // 16, 16).T
        gi_w = np.tile(wrap, (8, 1)).astype(np.int16)

        cores.append(dict(q=q, s=s, vgq=vgq, vo=vo, gi=gi_w))
    return cores, OV


def _build(OV):
    import concourse.bacc as bacc
    import concourse.mybir as mybir
    from concourse import tile

    f32 = mybir.dt.float32
    bf16 = mybir.dt.bfloat16
    fp8 = mybir.dt.float8e4
    i16 = mybir.dt.int16
    NST = NCH_B + 2 * OV
    NIDX = BLKS * OV * P

    nc = bacc.Bacc(
        None,
        target_bir_lowering=False,
        num_devices=C,
        dynamic_dma_scratch_size=1 << 16,
    )

    featT_d = nc.dram_tensor("featT", [IN_CH, NPAD], bf16, kind="ExternalInput")
    w_d = nc.dram_tensor("w", [IN_CH, OUT_CH], bf16, kind="ExternalInput")
    bias_d = nc.dram_tensor("bias", [P, OUT_CH], f32, kind="ExternalInput")
    q_d = nc.dram_tensor("q", [P, NSLOT], fp8, kind="ExternalInput")
    s_d = nc.dram_tensor("s", [P, BLKS * NST * P], fp8, kind="ExternalInput")
    vgq_d = nc.dram_tensor("vgq", [P, NBP * NCH_B * QB], bf16,
                           kind="ExternalInput")
    vo_d = nc.dram_tensor("vo", [P, BLKS * OV], bf16, kind="ExternalInput")
    gi_d = nc.dram_tensor("gi", [P, NIDX // 16], i16, kind="ExternalInput")
    out_d = nc.dram_tensor("out", [NPAD, OUT_CH], f32, kind="ExternalOutput")

    xsh_d = nc.dram_tensor("x_shard", [P, BLKP * OUT_CH], bf16)
    xA_d = nc.dram_tensor("xA", [C * P, BLKP * OUT_CH], bf16,
                          addr_space="Shared")
    xB_d = nc.dram_tensor("xB", [C * P, BLKP * OUT_CH], bf16,
                          addr_space="Shared")
    arena_d = nc.dram_tensor("arena", [NSLOT, OUT_CH], bf16)

    # stage-A chunk -> (gsb_lo, split width)
    chunks = []
    for k in range(NCHA):
        glo = (k * P) // RSZ
        w1 = min((glo + 1) * RSZ - k * P, P)
        chunks.append((glo, w1))

    with tile.TileContext(nc) as tc:
        with (
            tc.tile_pool(name="const", bufs=1) as cpool,
            tc.tile_pool(name="x", bufs=1) as xpool,
            tc.tile_pool(name="o", bufs=3) as opool,
            tc.tile_pool(name="psA", bufs=2, space="PSUM") as ppA,
            tc.tile_pool(name="psB", bufs=3, space="PSUM") as ppB,
        ):
            w_sb = cpool.tile([P, IN_CH // P, OUT_CH], bf16)
            bias_sb = cpool.tile([P, OUT_CH], f32)
            gi_sb = cpool.tile([P, NIDX // 16], i16)
            vgq_sb = cpool.tile([P, NBP * NCH_B * QB], bf16)
            vo_sb = cpool.tile([P, BLKS * OV], bf16)
            x_sb = xpool.tile([P, GSBP, OUT_CH], bf16)
            x1_sb = cpool.tile([P, BLKP, OUT_CH], bf16)

            nc.sync.dma_start(w_sb[:], w_d[:].rearrange("(k p) c -> p k c", p=P))
            nc.sync.dma_start(bias_sb[:], bias_d[:])
            nc.sync.dma_start(gi_sb[:], gi_d[:])
            nc.sync.dma_start(vgq_sb[:], vgq_d[:])
            nc.sync.dma_start(vo_sb[:], vo_d[:])
            nc.vector.memset(x1_sb[:, BLKS, :], 0.0)

            def rr_copy(i, out, in_):
                if i % 2 == 0:
                    nc.vector.tensor_copy(out, in_)
                else:
                    nc.scalar.copy(out, in_)

            # --- projection: x0 = features @ W (p-major into x1_sb) ---
            GRP = 7
            with tc.tile_pool(name="feat", bufs=2) as fpool:
                for gg in range(BLKS // GRP):
                    feat_sb = fpool.tile([P, IN_CH // P, GRP * P], bf16, tag="f")
                    nc.sync.dma_start(
                        feat_sb[:],
                        featT_d[:, gg * GRP * P:(gg + 1) * GRP * P].rearrange(
                            "(k p) n -> p k n", p=P),
                    )
                    for bb in range(GRP):
                        b = gg * GRP + bb
                        ps = ppB.tile([P, OUT_CH], f32, tag="psB")
                        for k in range(IN_CH // P):
                            nc.tensor.matmul(
                                ps[:],
                                feat_sb[:, k, bb * P:(bb + 1) * P],
                                w_sb[:, k, :],
                                start=(k == 0),
                                stop=(k == IN_CH // P - 1),
                            )
                        rr_copy(b, x1_sb[:, b, :], ps[:])
                nc.sync.dma_start(
                    xsh_d[:], x1_sb[:].rearrange("p g c -> p (g c)"))

            sp1 = tc.tile_pool(name="q", bufs=2)
            qpool = sp1.__enter__()
            sp2 = tc.tile_pool(name="stg", bufs=3)
            spool = sp2.__enter__()
            sp3 = tc.tile_pool(name="G", bufs=2)
            gpool = sp3.__enter__()
            sp4 = tc.tile_pool(name="S", bufs=2)
            sspool = sp4.__enter__()
            sp5 = tc.tile_pool(name="gov", bufs=1)
            govpool = sp5.__enter__()

            def allgather(dst):
                nc.gpsimd.collective_compute(
                    "AllGather",
                    mybir.AluOpType.bypass,
                    replica_groups=[list(range(C))],
                    ins=[xsh_d[:]],
                    outs=[dst[:]],
                )

            def spmm(src, last):
                # replica load, one slab per source core (overlaps stage A)
                for a in range(C):
                    eng = nc.sync if a % 2 == 0 else nc.scalar
                    eng.dma_start(
                        x_sb[:, a * BLKP:(a + 1) * BLKP, :].rearrange(
                            "p g c -> p (g c)"),
                        src[a * P:(a + 1) * P, :])
                # overflow gather (256B pairs of b-adjacent rows)
                gov = govpool.tile([P, BLKS * OV, 2 * OUT_CH], bf16, tag="gov")
                nc.gpsimd.dma_gather(
                    gov[:],
                    src[:].rearrange("r (q w) -> (r q) w", q=BLKP // 2),
                    gi_sb[:], NIDX, NIDX, 2 * OUT_CH,
                    single_packet=False)
                nc.vector.tensor_tensor(
                    gov[:], gov[:],
                    vo_sb[:].unsqueeze(2).broadcast_to(
                        [P, BLKS * OV, 2 * OUT_CH]),
                    mybir.AluOpType.mult)

                # --- stage A: AIT chunks per iteration ---
                qt = None
                for it in range((NCHA + AIT - 1) // AIT):
                    k0 = it * AIT
                    nch = min(AIT, NCHA - k0)
                    if k0 % QIT == 0:
                        qt = qpool.tile([P, QIT, P], fp8, tag="q")
                        nq = min(QIT * P, NSLOT - k0 * P)
                        nc.sync.dma_start(
                            qt[:].rearrange("p a q -> p (a q)")[:, :nq],
                            q_d[:, k0 * P:k0 * P + nq])
                    ps = ppA.tile([P, AIT * OUT_CH], f32, tag="psA")
                    for j in range(nch):
                        k = k0 + j
                        glo, w1 = chunks[k]
                        qoff = k % QIT
                        pv = ps[:, j * OUT_CH:(j + 1) * OUT_CH]
                        nc.tensor.matmul(
                            pv[:w1, :], qt[:, qoff, :w1],
                            x_sb[:, glo, :], start=True, stop=True)
                        if w1 < P:
                            nc.tensor.matmul(
                                pv[w1:, :], qt[:, qoff, w1:],
                                x_sb[:, glo + 1, :], start=True, stop=True)
                    stg = spool.tile([P, AIT, OUT_CH], bf16, tag="stg")
                    rr_copy(it, stg[:, :nch, :],
                            ps[:, :nch * OUT_CH].rearrange(
                                "p (a c) -> p a c", a=nch))
                    eng = nc.sync if it % 2 == 0 else nc.scalar
                    eng.dma_start(
                        arena_d[k0 * P:k0 * P + nch * P, :].rearrange(
                            "(a p) c -> p a c", p=P),
                        stg[:, :nch, :])

                # --- stage B: QB blocks per load ---
                arena_v = arena_d[:].rearrange(
                    "(g p x) c -> p g x c", g=NCH_B, p=P, x=64)
                for bp in range(NBP):
                    nb = min(QB, BLKS - QB * bp)
                    G2 = gpool.tile([P, NCH_B, QB, OUT_CH], bf16, tag="G")
                    nc.scalar.dma_start(
                        G2[:, :, :nb, :],
                        arena_v[:, :, QB * bp:QB * bp + nb, :])
                    nc.vector.tensor_tensor(
                        G2[:], G2[:],
                        vgq_sb[:, bp * NCH_B * QB:(bp + 1) * NCH_B * QB]
                        .rearrange("p (g i) -> p g i", g=NCH_B)
                        .unsqueeze(3).broadcast_to([P, NCH_B, QB, OUT_CH]),
                        mybir.AluOpType.mult)
                    S2 = sspool.tile([P, QB * NST, P], fp8, tag="S")
                    nc.scalar.dma_start(
                        S2[:, :nb * NST, :],
                        s_d[:, QB * bp * NST * P:
                            (QB * bp + nb) * NST * P].rearrange(
                            "p (s q) -> p s q", s=nb * NST))
                    for i in range(nb):
                        b = QB * bp + i
                        ps = ppB.tile([P, OUT_CH], f32, tag="psB")
                        for G1 in range(NCH_B):
                            nc.tensor.matmul(
                                ps[:], S2[:, i * NST + G1, :],
                                G2[:, G1, i, :],
                                start=(G1 == 0), stop=False)
                        for k in range(OV):
                            nc.tensor.matmul(
                                ps[:], S2[:, i * NST + NCH_B + 2 * k, :],
                                gov[:, b * OV + k, 0:OUT_CH],
                                start=False, stop=False)
                            nc.tensor.matmul(
                                ps[:], S2[:, i * NST + NCH_B + 2 * k + 1, :],
                                gov[:, b * OV + k, OUT_CH:2 * OUT_CH],
                                start=False, stop=(k == OV - 1))
                        if last:
                            o = opool.tile([P, OUT_CH], f32, tag="o")
                            nc.vector.tensor_tensor(
                                o[:], ps[:], bias_sb[:], mybir.AluOpType.add)
                            nc.scalar.dma_start(
                                out_d[b * P:(b + 1) * P, :], o[:])
                        else:
                            rr_copy(b, x1_sb[:, b, :], ps[:])
                if not last:
                    nc.sync.dma_start(
                        xsh_d[:], x1_sb[:].rearrange("p g c -> p (g c)"))

            allgather(xA_d)
            spmm(xA_d, last=False)
            allgather(xB_d)
            spmm(xB_d, last=True)

            sp5.__exit__(None, None, None)
            sp4.__exit__(None, None, None)
            sp3.__exit__(None, None, None)
            sp2.__exit__(None, None, None)
            sp1.__exit__(None, None, None)

    nc.compile()
    return nc


LAST_RESULT = None


def kernel(adj_indices, adj_values, features, weight, bias):
    global LAST_RESULT
    from concourse.bass_utils import run_bass_kernel_spmd

    cores, OV = _prep(np.asarray(adj_indices), np.asarray(adj_values))

    if OV not in _CACHE:
        _CACHE[OV] = _build(OV)
    nc = _CACHE[OV]

    features = np.asarray(features, np.float32)
    weight = np.ascontiguousarray(
        np.asarray(weight, np.float32).astype(ml_dtypes.bfloat16))
    bias128 = np.tile(np.asarray(bias, np.float32).reshape(1, OUT_CH), (P, 1))

    in_maps = []
    for c in range(C):
        featT = np.zeros((IN_CH, NPAD), ml_dtypes.bfloat16)
        featT[:, :NSHARD] = (
            features[c * NSHARD:(c + 1) * NSHARD].T.astype(ml_dtypes.bfloat16))
        h = cores[c]
        in_maps.append({
            "featT": featT,
            "w": weight,
            "bias": bias128,
            "q": h["q"],
            "s": h["s"],
            "vgq": h["vgq"],
            "vo": h["vo"],
            "gi": np.ascontiguousarray(h["gi"]),
        })

    res = run_bass_kernel_spmd(nc, in_maps, core_ids=list(range(C)))
    LAST_RESULT = res

    out = np.concatenate(
        [res.results[c]["out"][:NSHARD] for c in range(C)], axis=0)
    return out


# revision 3
# speedup vs baseline: 1.0570x; 1.0168x over previous
"""DenseNGCN on 8 trn2 cores — mailbox (two-level one-hot matmul) design.

  x = features @ W; twice: x <- A @ x; out = x + bias.

Per pass, per core (dst-sharded, x replicated via AllGather in p-major
layout [128, 50, 64] per core):
  Stage A: packed mailbox of 160k slots (400 gsb-regions x 400: slot =
    gsb*400 + t*50 + bl, T=8): 1250 chunk matmuls (+~300 straddle splits)
    route source rows via fp8 0/1 Q one-hots streamed from HBM; psum
    chunks land contiguously in an HBM arena.
  Stage B: per dst block bl: strided DMA pulls [p = g0*8+t, G1] chunks
    (stride 50 rows); fp8 0/1 S one-hots streamed; vals applied to G via
    one batched broadcast multiply per 4 blocks; 25+2*OV matmuls per block
    accumulate in PSUM.
  Overflow edges beyond T=8 per (gsb, bl): one dma_gather of 256B pairs
    (b-adjacent rows in p-major layout) per pass feeds 2*OV extra matmuls
    per block (parity-split halves).
"""

import numpy as np
import ml_dtypes

N = 50000
E = 800000
IN_CH = 512
OUT_CH = 64
C = 8
P = 128
NSHARD = 6250
BLKS = 49            # real dst blocks per core
BLKP = 50            # padded p-major slab blocks (b=49 dummy)
NPAD = 6272
GSBP = C * BLKP      # 400 padded global src blocks
T = 8
RSZ = 512            # slots per gsb region (r = t*64 + bl)
NSLOT = GSBP * RSZ   # 160000
NCHA = NSLOT // P    # 1250 stage-A chunks
NCH_B = GSBP // 16   # 25
QB = 4               # stage-B blocks per load batch
NBP = (BLKS + QB - 1) // QB
AIT = 16             # stage-A chunks per iteration
QIT = 32             # chunks per Q load

_CACHE = {}


def _prep(adj_indices, adj_values):
    row = adj_indices[0].astype(np.int64)
    col = adj_indices[1].astype(np.int64)
    val = adj_values.astype(np.float32)

    cd = row // NSHARD
    bl = (row % NSHARD) // P
    dl = (row % NSHARD) % P
    cs = col // NSHARD
    lsrc = col % NSHARD
    bsrc = lsrc // P
    sl = lsrc % P
    gsb = cs * BLKP + bsrc

    percore = []
    max_ovf = 0
    for c in range(C):
        m = np.where(cd == c)[0]
        order = m[np.lexsort((m, bl[m], gsb[m]))]
        pg = gsb[order] * BLKS + bl[order]
        _, start_idx, cnt = np.unique(pg, return_index=True,
                                      return_counts=True)
        rank = np.arange(len(order)) - np.repeat(start_idx, cnt)
        main = rank < T
        ovf_bl = np.bincount(bl[order[~main]], minlength=BLKS)
        max_ovf = max(max_ovf, int(ovf_bl.max()))
        percore.append((order, rank, main))
    OV = max(1, (max_ovf + P - 1) // P)
    NST = NCH_B + 2 * OV

    f8 = ml_dtypes.float8_e4m3
    bf = ml_dtypes.bfloat16
    cores = []
    for c in range(C):
        order, rank, main = percore[c]
        e_main = order[main]
        t_main = rank[main]

        # packed Q: [128, NSLOT] fp8; slot = gsb*400 + t*50 + bl
        slot = gsb[e_main] * RSZ + t_main * 64 + bl[e_main]
        q = np.zeros((P, NSLOT), f8)
        q[sl[e_main], slot] = 1.0

        # S: [128, BLKS*NST*128] fp8 0/1; partition p = g0*8+t; col = dl
        s = np.zeros((P, BLKS * NST * P), f8)
        vgq = np.zeros((P, NBP * NCH_B * QB), bf)
        vo = np.zeros((P, BLKS * OV), bf)
        g0 = gsb[e_main] % 16
        G1 = gsb[e_main] // 16
        p_b = g0 * 8 + t_main
        s[p_b, (bl[e_main] * NST + G1) * P + dl[e_main]] = 1.0
        # vgq layout: (p, bp, G1, i)
        vgq[p_b, ((bl[e_main] // QB) * NCH_B + G1) * QB
            + bl[e_main] % QB] = val[e_main]

        e_ovf = order[~main]
        o_order = e_ovf[np.argsort(bl[e_ovf], kind="stable")]
        ob = bl[o_order]
        _, ostart, ocnt = np.unique(ob, return_index=True, return_counts=True)
        orank = np.zeros(len(o_order), np.int64)
        if len(o_order):
            orank = np.arange(len(o_order)) - np.repeat(ostart, ocnt)
        ch = ob * OV + orank // P
        pp_ = orank % P
        gi_ovf = np.zeros((BLKS * OV, P), np.int64)
        gi_ovf[ch, pp_] = ((cs[o_order] * P + sl[o_order]) * (BLKP // 2)
                           + bsrc[o_order] // 2)
        par = bsrc[o_order] % 2
        se = (ob * NST + NCH_B + 2 * (orank // P)) * P
        s[pp_[par == 0], se[par == 0] + dl[o_order[par == 0]]] = 1.0
        s[pp_[par == 1], se[par == 1] + P + dl[o_order[par == 1]]] = 1.0
        vo[pp_, ch] = val[o_order]

        nidx = BLKS * OV * P
        wrap = gi_ovf.reshape(-1).reshape(nidx // 16, 16).T
        gi_w = np.tile(wrap, (8, 1)).astype(np.int16)

        cores.append(dict(q=q, s=s, vgq=vgq, vo=vo, gi=gi_w))
    return cores, OV


def _build(OV):
    import concourse.bacc as bacc
    import concourse.mybir as mybir
    from concourse import tile

    f32 = mybir.dt.float32
    bf16 = mybir.dt.bfloat16
    fp8 = mybir.dt.float8e4
    i16 = mybir.dt.int16
    NST = NCH_B + 2 * OV
    NIDX = BLKS * OV * P

    nc = bacc.Bacc(
        None,
        target_bir_lowering=False,
        num_devices=C,
        dynamic_dma_scratch_size=1 << 16,
    )

    featT_d = nc.dram_tensor("featT", [IN_CH, NPAD], bf16, kind="ExternalInput")
    w_d = nc.dram_tensor("w", [IN_CH, OUT_CH], bf16, kind="ExternalInput")
    bias_d = nc.dram_tensor("bias", [P, OUT_CH], f32, kind="ExternalInput")
    q_d = nc.dram_tensor("q", [P, NSLOT], fp8, kind="ExternalInput")
    s_d = nc.dram_tensor("s", [P, BLKS * NST * P], fp8, kind="ExternalInput")
    vgq_d = nc.dram_tensor("vgq", [P, NBP * NCH_B * QB], bf16,
                           kind="ExternalInput")
    vo_d = nc.dram_tensor("vo", [P, BLKS * OV], bf16, kind="ExternalInput")
    gi_d = nc.dram_tensor("gi", [P, NIDX // 16], i16, kind="ExternalInput")
    out_d = nc.dram_tensor("out", [NPAD, OUT_CH], f32, kind="ExternalOutput")

    xsh_d = nc.dram_tensor("x_shard", [P, BLKP * OUT_CH], bf16)
    xA_d = nc.dram_tensor("xA", [C * P, BLKP * OUT_CH], bf16,
                          addr_space="Shared")
    xB_d = nc.dram_tensor("xB", [C * P, BLKP * OUT_CH], bf16,
                          addr_space="Shared")
    arena_d = nc.dram_tensor("arena", [NSLOT, OUT_CH], bf16)

    # stage-A chunk -> (gsb_lo, split width)
    chunks = []
    for k in range(NCHA):
        glo = (k * P) // RSZ
        w1 = min((glo + 1) * RSZ - k * P, P)
        chunks.append((glo, w1))

    with tile.TileContext(nc) as tc:
        with (
            tc.tile_pool(name="const", bufs=1) as cpool,
            tc.tile_pool(name="x", bufs=1) as xpool,
            tc.tile_pool(name="o", bufs=3) as opool,
            tc.tile_pool(name="psA", bufs=2, space="PSUM") as ppA,
            tc.tile_pool(name="psB", bufs=3, space="PSUM") as ppB,
        ):
            w_sb = cpool.tile([P, IN_CH // P, OUT_CH], bf16)
            bias_sb = cpool.tile([P, OUT_CH], f32)
            gi_sb = cpool.tile([P, NIDX // 16], i16)
            vgq_sb = cpool.tile([P, NBP * NCH_B * QB], bf16)
            vo_sb = cpool.tile([P, BLKS * OV], bf16)
            x_sb = xpool.tile([P, GSBP, OUT_CH], bf16)
            x1_sb = cpool.tile([P, BLKP, OUT_CH], bf16)
            gate_sb = cpool.tile([P, 1, OUT_CH], bf16)

            nc.sync.dma_start(w_sb[:], w_d[:].rearrange("(k p) c -> p k c", p=P))
            nc.sync.dma_start(bias_sb[:], bias_d[:])
            nc.sync.dma_start(gi_sb[:], gi_d[:])
            nc.sync.dma_start(vgq_sb[:], vgq_d[:])
            nc.sync.dma_start(vo_sb[:], vo_d[:])
            nc.vector.memset(x1_sb[:, BLKS, :], 0.0)

            def rr_copy(i, out, in_):
                if i % 2 == 0:
                    nc.vector.tensor_copy(out, in_)
                else:
                    nc.scalar.copy(out, in_)

            # --- projection: x0 = features @ W (p-major into x1_sb) ---
            GRP = 7
            with tc.tile_pool(name="feat", bufs=2) as fpool:
                for gg in range(BLKS // GRP):
                    feat_sb = fpool.tile([P, IN_CH // P, GRP * P], bf16, tag="f")
                    nc.sync.dma_start(
                        feat_sb[:],
                        featT_d[:, gg * GRP * P:(gg + 1) * GRP * P].rearrange(
                            "(k p) n -> p k n", p=P),
                    )
                    for bb in range(GRP):
                        b = gg * GRP + bb
                        ps = ppB.tile([P, OUT_CH], f32, tag="psB")
                        for k in range(IN_CH // P):
                            nc.tensor.matmul(
                                ps[:],
                                feat_sb[:, k, bb * P:(bb + 1) * P],
                                w_sb[:, k, :],
                                start=(k == 0),
                                stop=(k == IN_CH // P - 1),
                            )
                        rr_copy(b, x1_sb[:, b, :], ps[:])
                nc.sync.dma_start(
                    xsh_d[:], x1_sb[:].rearrange("p g c -> p (g c)"))

            sp1 = tc.tile_pool(name="q", bufs=2)
            qpool = sp1.__enter__()
            sp2 = tc.tile_pool(name="stg", bufs=3)
            spool = sp2.__enter__()
            sp3 = tc.tile_pool(name="G", bufs=2)
            gpool = sp3.__enter__()
            sp4 = tc.tile_pool(name="S", bufs=2)
            sspool = sp4.__enter__()
            sp5 = tc.tile_pool(name="gov", bufs=1)
            govpool = sp5.__enter__()

            def allgather(dst):
                nc.gpsimd.collective_compute(
                    "AllGather",
                    mybir.AluOpType.bypass,
                    replica_groups=[list(range(C))],
                    ins=[xsh_d[:]],
                    outs=[dst[:]],
                )

            def spmm(src, last):
                # replica load, one slab per source core (overlaps stage A)
                for a in range(C):
                    eng = nc.sync if a % 2 == 0 else nc.scalar
                    eng.dma_start(
                        x_sb[:, a * BLKP:(a + 1) * BLKP, :].rearrange(
                            "p g c -> p (g c)"),
                        src[a * P:(a + 1) * P, :])
                # --- stage A: AIT chunks per iteration ---
                qt = None
                for it in range((NCHA + AIT - 1) // AIT):
                    k0 = it * AIT
                    nch = min(AIT, NCHA - k0)
                    if k0 % QIT == 0:
                        qt = qpool.tile([P, QIT, P], fp8, tag="q")
                        nq = min(QIT * P, NSLOT - k0 * P)
                        nc.sync.dma_start(
                            qt[:].rearrange("p a q -> p (a q)")[:, :nq],
                            q_d[:, k0 * P:k0 * P + nq])
                    ps = ppA.tile([P, AIT * OUT_CH], f32, tag="psA")
                    for j in range(nch):
                        k = k0 + j
                        glo, w1 = chunks[k]
                        qoff = k % QIT
                        pv = ps[:, j * OUT_CH:(j + 1) * OUT_CH]
                        nc.tensor.matmul(
                            pv[:w1, :], qt[:, qoff, :w1],
                            x_sb[:, glo, :], start=True, stop=True)
                        if w1 < P:
                            nc.tensor.matmul(
                                pv[w1:, :], qt[:, qoff, w1:],
                                x_sb[:, glo + 1, :], start=True, stop=True)
                    stg = spool.tile([P, AIT, OUT_CH], bf16, tag="stg")
                    rr_copy(it, stg[:, :nch, :],
                            ps[:, :nch * OUT_CH].rearrange(
                                "p (a c) -> p a c", a=nch))
                    eng = nc.sync if it % 2 == 0 else nc.scalar
                    eng.dma_start(
                        arena_d[k0 * P:k0 * P + nch * P, :].rearrange(
                            "(a p) c -> p a c", p=P),
                        stg[:, :nch, :])
                    if it == 30:
                        # gates the overflow gather: gpsimd waits for
                        # mid-stage-A progress before flooding the DMA
                        # rings with its 6k gather descriptors
                        nc.gpsimd.tensor_copy(gate_sb[:], stg[:, :1, :])

                # overflow gather (256B pairs of b-adjacent rows);
                # issued after stage A so its descriptors don't stall the
                # stage-A Q/x streams, overlaps with early stage B
                gov = govpool.tile([P, BLKS * OV, 2 * OUT_CH], bf16, tag="gov")
                nc.gpsimd.dma_gather(
                    gov[:],
                    src[:].rearrange("r (q w) -> (r q) w", q=BLKP // 2),
                    gi_sb[:], NIDX, NIDX, 2 * OUT_CH,
                    single_packet=False)
                nc.vector.tensor_tensor(
                    gov[:], gov[:],
                    vo_sb[:].unsqueeze(2).broadcast_to(
                        [P, BLKS * OV, 2 * OUT_CH]),
                    mybir.AluOpType.mult)

                # --- stage B: QB blocks per load ---
                arena_v = arena_d[:].rearrange(
                    "(g p x) c -> p g x c", g=NCH_B, p=P, x=64)
                for bp in range(NBP):
                    nb = min(QB, BLKS - QB * bp)
                    G2 = gpool.tile([P, NCH_B, QB, OUT_CH], bf16, tag="G")
                    nc.scalar.dma_start(
                        G2[:, :, :nb, :],
                        arena_v[:, :, QB * bp:QB * bp + nb, :])
                    nc.vector.tensor_tensor(
                        G2[:], G2[:],
                        vgq_sb[:, bp * NCH_B * QB:(bp + 1) * NCH_B * QB]
                        .rearrange("p (g i) -> p g i", g=NCH_B)
                        .unsqueeze(3).broadcast_to([P, NCH_B, QB, OUT_CH]),
                        mybir.AluOpType.mult)
                    S2 = sspool.tile([P, QB * NST, P], fp8, tag="S")
                    nc.scalar.dma_start(
                        S2[:, :nb * NST, :],
                        s_d[:, QB * bp * NST * P:
                            (QB * bp + nb) * NST * P].rearrange(
                            "p (s q) -> p s q", s=nb * NST))
                    for i in range(nb):
                        b = QB * bp + i
                        ps = ppB.tile([P, OUT_CH], f32, tag="psB")
                        for G1 in range(NCH_B):
                            nc.tensor.matmul(
                                ps[:], S2[:, i * NST + G1, :],
                                G2[:, G1, i, :],
                                start=(G1 == 0), stop=False)
                        for k in range(OV):
                            nc.tensor.matmul(
                                ps[:], S2[:, i * NST + NCH_B + 2 * k, :],
                                gov[:, b * OV + k, 0:OUT_CH],
                                start=False, stop=False)
                            nc.tensor.matmul(
                                ps[:], S2[:, i * NST + NCH_B + 2 * k + 1, :],
                                gov[:, b * OV + k, OUT_CH:2 * OUT_CH],
                                start=False, stop=(k == OV - 1))
                        if last:
                            o = opool.tile([P, OUT_CH], f32, tag="o")
                            nc.vector.tensor_tensor(
                                o[:], ps[:], bias_sb[:], mybir.AluOpType.add)
                            nc.scalar.dma_start(
                                out_d[b * P:(b + 1) * P, :], o[:])
                        else:
                            rr_copy(b, x1_sb[:, b, :], ps[:])
                if not last:
                    nc.sync.dma_start(
                        xsh_d[:], x1_sb[:].rearrange("p g c -> p (g c)"))

            allgather(xA_d)
            spmm(xA_d, last=False)
            allgather(xB_d)
            spmm(xB_d, last=True)

            sp5.__exit__(None, None, None)
            sp4.__exit__(None, None, None)
            sp3.__exit__(None, None, None)
            sp2.__exit__(None, None, None)
            sp1.__exit__(None, None, None)

    nc.compile()
    return nc


LAST_RESULT = None


def kernel(adj_indices, adj_values, features, weight, bias):
    global LAST_RESULT
    from concourse.bass_utils import run_bass_kernel_spmd

    cores, OV = _prep(np.asarray(adj_indices), np.asarray(adj_values))

    if OV not in _CACHE:
        _CACHE[OV] = _build(OV)
    nc = _CACHE[OV]

    features = np.asarray(features, np.float32)
    weight = np.ascontiguousarray(
        np.asarray(weight, np.float32).astype(ml_dtypes.bfloat16))
    bias128 = np.tile(np.asarray(bias, np.float32).reshape(1, OUT_CH), (P, 1))

    in_maps = []
    for c in range(C):
        featT = np.zeros((IN_CH, NPAD), ml_dtypes.bfloat16)
        featT[:, :NSHARD] = (
            features[c * NSHARD:(c + 1) * NSHARD].T.astype(ml_dtypes.bfloat16))
        h = cores[c]
        in_maps.append({
            "featT": featT,
            "w": weight,
            "bias": bias128,
            "q": h["q"],
            "s": h["s"],
            "vgq": h["vgq"],
            "vo": h["vo"],
            "gi": np.ascontiguousarray(h["gi"]),
        })

    res = run_bass_kernel_spmd(nc, in_maps, core_ids=list(range(C)))
    LAST_RESULT = res

    out = np.concatenate(
        [res.results[c]["out"][:NSHARD] for c in range(C)], axis=0)
    return out
